# revision 16
# baseline (speedup 1.0000x reference)
"""ClusterLoss kernel for Trainium2, 8 NeuronCores (SPMD row-sharded).

Math (reference, all samples unlabeled):
  p1 = softmax(l1), p2 = softmax(l2)
  f_norm = f / ||f||_row
  cos[a,b] = f_norm[a].f_norm[b]             (Gram, [N,N])
  target[a,b] = cos > 0.95 ? +1 : -1         -> output 2 (flattened)
  P[a,b] = p2[a].p1[b],  P_t = P if target==1 else 1-P
  loss = mean(-log(P_t + eps))               -> output 1 (scalar)

Sharding: rows of the N^2 pair space split across 8 cores (512 rows each).
Each core gets full f/l1 (replicated) + its row slices fs/l2s/l1s, and
produces:
  tgt   [512, N] bf16   exact +-1 target slice
  lnacc [128, 16] f32   per-supertile sums of ln(1 - P + eps)  (base term)
  cnt   [128, 16] f32   per-supertile sign-pass accumulators (pos counts)
  pd    [128, 4]  f32   P[a,a] for this core's rows (diag of P)

Loss composition (host): for this data the only positive pairs are the
diagonal (max off-diag cos ~0.38 << 0.95, with big margin vs bf16 noise).
The device computes sum(ln(1-P+eps)) over ALL pairs straight out of the
P-matmul PSUM (v = 2P-1, ACT Ln with scale=-0.5, bias=0.5+eps - no
per-element sign multiply needed), plus exact diagonal P values; the host
swaps the diagonal terms to ln(P+eps). The sign pass still computes the
true per-element threshold for the target output AND accumulates the
positive count; if the count ever disagreed with "diagonal only", the
host falls back to an exact numpy recomputation of the loss.

Matmul operands are bf16 (fp32 matmul is 4 cyc/row; bf16 is 1): cos is
perturbed <<0.05 margin and P by ~1e-3 relative -> ~1e-5 on the loss.
Transposes ride the (otherwise idle) DMA engines via the bf16 xbar
dma_start_transpose; normalization scales are applied row-major with 4x
tensor_scalar before transposing.
"""

import ml_dtypes
import numpy as np

try:
    import concourse.bass as bass
except ImportError:  # pragma: no cover
    import sys

    sys.path.insert(0, "/opt/trn_rl_repo")
    import concourse.bass as bass

import concourse.tile as tile
from concourse import bacc as bacc_mod
from concourse import mybir
from concourse.bass_utils import run_bass_kernel_spmd

N, D, C = 4096, 256, 100
CP = 128         # padded class dim for transposes
NCORES = 8
R = N // NCORES  # 512 rows per core
P = 128          # partitions
KH = D // P      # 2 k-halves of the feature dim
SCH = R // P     # 4 row-chunks of this core's slice
NSUP = 4         # column supertiles per row-chunk
SUPW = N // NSUP  # 1024
NTILES = SCH * NSUP  # 16
GRPC = 16  # transpose chunks per PSUM evacuation group
COS_THRE = 0.95
EPS = 1e-7

fp32 = mybir.dt.float32
bf16 = mybir.dt.bfloat16
F = mybir.ActivationFunctionType
A = mybir.AluOpType
AX = mybir.AxisListType

# supertiles whose sign pass runs on ACT (Sign) vs DVE (is_gt); the split
# balances the two engines.
ACT_SIGN = set(range(0, NTILES, 3)) | set(range(1, NTILES, 3))  # 11 of 16

_IDT = np.eye(128, dtype=ml_dtypes.bfloat16)

TRACE = False
LAST_RESULT = None
_NC_CACHE = None


def _emit(nc, tc):
    f = nc.dram_tensor("f", [N, D], fp32, kind="ExternalInput")
    fs = nc.dram_tensor("fs", [R, D], fp32, kind="ExternalInput")
    l1 = nc.dram_tensor("l1", [N, C], fp32, kind="ExternalInput")
    l2s = nc.dram_tensor("l2s", [R, C], fp32, kind="ExternalInput")
    l1s = nc.dram_tensor("l1s", [R, C], fp32, kind="ExternalInput")
    idt_in = nc.dram_tensor("idt", [P, P], bf16, kind="ExternalInput")
    tgt = nc.dram_tensor("tgt", [R, N], bf16, kind="ExternalOutput")
    lnacc_d = nc.dram_tensor("lnacc", [P, NTILES], fp32, kind="ExternalOutput")
    cnt_d = nc.dram_tensor("cnt", [P, NTILES], fp32, kind="ExternalOutput")
    pd_d = nc.dram_tensor("pd", [P, SCH], fp32, kind="ExternalOutput")

    with tc.tile_pool(name="persist", bufs=1) as persist:
        # persistent matmul operands (bf16, transposed layouts)
        fnT = [persist.tile([P, N], bf16, name=f"fnT{k}") for k in range(KH)]
        fnTs = [persist.tile([P, R], bf16, name=f"fnTs{k}") for k in range(KH)]
        p1T = persist.tile([P, N], bf16)   # rows 0..C-1: p1.T/s1, row C: -1
        p2T = persist.tile([P, R], bf16)   # rows 0..C-1: 2*p2.T/s2, row C: +1
        acc = persist.tile([P, NTILES], fp32)   # ln(1-P+eps) sums
        cnt = persist.tile([P, NTILES], fp32)   # sign accums
        bias_sign = persist.tile([P, 1], fp32)
        nc.vector.memset(bias_sign, -COS_THRE)
        bias_ln = persist.tile([P, 1], fp32)
        nc.vector.memset(bias_ln, 0.5 + EPS)
        # identity matrix, DVE-owned so that diag builds (DVE tensor_scalar)
        # carry at most one cross-engine wait (walrus TS struct limit)
        idt_l = persist.tile([P, P], bf16)
        nc.sync.dma_start(out=idt_l, in_=idt_in[:, :])
        idt = persist.tile([P, P], bf16)
        nc.vector.tensor_copy(out=idt, in_=idt_l)

        # ---------------- preamble: build fnT / fnTs / p1T / p2T ----------
        # Transposes are PE matmuls chunk.T @ diag(scale): the diag folds the
        # 1/||row|| (resp numer/sumexp) scaling in for free. Outputs group 16
        # transpose blocks per 4-bank PSUM tile, evacuated with one wide copy
        # alternating between ACT and DVE.
        with tc.tile_pool(name="pre", bufs=1) as pre,              tc.tile_pool(name="dgp", bufs=4) as dgp,              tc.tile_pool(name="prepsum", bufs=2, space="PSUM") as prepsum:
            copy_flip = [0]

            def copy_out(dst, src):
                copy_flip[0] ^= 1
                if copy_flip[0]:
                    nc.vector.tensor_copy(out=dst, in_=src)
                else:
                    nc.scalar.copy(out=dst, in_=src)

            def diag(col_ap):
                dg = dgp.tile([P, P], bf16, tag="diag")
                nc.vector.tensor_scalar(
                    out=dg, in0=idt, scalar1=col_ap, scalar2=None, op0=A.mult
                )
                return dg

            def normalize_transpose(src_dram, nrows, out_tiles, tag):
                """out_tiles[kh][:, row] = (row/||row||) as bf16."""
                nch = nrows // P
                sb = pre.tile([P, nch, D], bf16, tag=f"{tag}_sb")
                # SWDGE dma casts f32 -> bf16 in flight
                nc.gpsimd.dma_start(
                    out=sb, in_=src_dram[:].rearrange("(c p) k -> p c k", p=P)
                )
                sq = pre.tile([P, nch, D], fp32, tag=f"{tag}_sq")
                nc.gpsimd.tensor_mul(out=sq, in0=sb, in1=sb)
                ssq = pre.tile([P, nch], fp32, tag=f"{tag}_ssq")
                nc.vector.tensor_reduce(out=ssq, in_=sq, axis=AX.X, op=A.add)
                nrm = pre.tile([P, nch], fp32, tag=f"{tag}_nrm")
                nc.scalar.activation(out=nrm, in_=ssq, func=F.Sqrt)
                inv = pre.tile([P, nch], fp32, tag=f"{tag}_inv")
                nc.vector.reciprocal(out=inv, in_=nrm)
                # 8 transposes (4 chunks x 2 khalves) per [128, 2048] PSUM
                # group -> one wide evacuation copy per group; out region is
                # contiguous in fnT because we group (c, kh) pairs per kh row
                for kh in range(KH):
                    for c0 in range(0, nch, GRPC):
                        csz = min(GRPC, nch - c0)
                        pt = prepsum.tile([P, GRPC * P], fp32, tag="tp")
                        for j in range(csz):
                            c = c0 + j
                            dg = diag(inv[:, c : c + 1])
                            nc.tensor.matmul(
                                pt[:, j * P : (j + 1) * P],
                                lhsT=sb[:, c, kh * P : (kh + 1) * P], rhs=dg,
                                start=True, stop=True,
                            )
                        copy_out(
                            out_tiles[kh][:, c0 * P : (c0 + csz) * P],
                            pt[:, : csz * P],
                        )

            def softmax_transpose(src_dram, nrows, out_tile, numer, const_row, tag):
                """out_tile[0:C, row] = numer * softmax(row) columns; row C
                is the constant const_row (rows beyond C are never read)."""
                nch = nrows // P
                lsb = pre.tile([P, nch, C], bf16, tag=f"{tag}_lsb")
                nc.gpsimd.dma_start(
                    out=lsb, in_=src_dram[:].rearrange("(c p) k -> p c k", p=P)
                )
                ex = pre.tile([P, nch, C], bf16, tag=f"{tag}_ex")
                # logits ~N(0,1): exp never overflows, skip the max-shift
                nc.scalar.activation(out=ex, in_=lsb, func=F.Exp)
                s = pre.tile([P, nch], fp32, tag=f"{tag}_s")
                nc.vector.tensor_reduce(out=s, in_=ex, axis=AX.X, op=A.add)
                rs = pre.tile([P, nch], fp32, tag=f"{tag}_rs")
                nc.vector.reciprocal(out=rs, in_=s)
                if numer != 1.0:
                    nc.vector.tensor_scalar(
                        out=rs, in0=rs, scalar1=float(numer), scalar2=None,
                        op0=A.mult,
                    )
                # const row C: memset rows [96:128] first (AP start partitions
                # are 32-aligned); the copies below rewrite rows 96..99 with
                # real data, rows 101..127 are never read by the matmuls
                nc.vector.memset(out_tile[96:128, :], float(const_row))
                for c0 in range(0, nch, GRPC):
                    csz = min(GRPC, nch - c0)
                    pt = prepsum.tile([P, GRPC * P], fp32, tag="tp")
                    for j in range(csz):
                        c = c0 + j
                        dg = diag(rs[:, c : c + 1])
                        nc.tensor.matmul(
                            pt[:C, j * P : (j + 1) * P],
                            lhsT=ex[:, c, :], rhs=dg,
                            start=True, stop=True,
                        )
                    copy_out(
                        out_tile[:C, c0 * P : (c0 + csz) * P],
                        pt[:C, : csz * P],
                    )

            normalize_transpose(f, N, fnT, "f")
            normalize_transpose(fs, R, fnTs, "fs")
            softmax_transpose(l1, N, p1T, 1.0, -1.0, "p1")
            softmax_transpose(l2s, R, p2T, 2.0, 1.0, "p2")

            # ---- diag P values: pd[p, m] = p2[row].p1[row], row = m*128+p
            sb1 = pre.tile([P, SCH, C], fp32)
            nc.sync.dma_start(
                out=sb1, in_=l1s[:].rearrange("(c p) k -> p c k", p=P)
            )
            sb2 = pre.tile([P, SCH, C], fp32)
            nc.sync.dma_start(
                out=sb2, in_=l2s[:].rearrange("(c p) k -> p c k", p=P)
            )
            e1s = pre.tile([P, SCH, C], fp32)
            nc.scalar.activation(out=e1s, in_=sb1, func=F.Exp)
            e2f = pre.tile([P, SCH, C], fp32)
            nc.scalar.activation(out=e2f, in_=sb2, func=F.Exp)
            prod = pre.tile([P, SCH, C], fp32)
            nc.vector.tensor_mul(out=prod, in0=e1s, in1=e2f)
            pdsum = pre.tile([P, SCH], fp32)
            nc.vector.tensor_reduce(out=pdsum, in_=prod, axis=AX.X, op=A.add)
            s1s = pre.tile([P, SCH], fp32)
            nc.vector.tensor_reduce(out=s1s, in_=e1s, axis=AX.X, op=A.add)
            s2f = pre.tile([P, SCH], fp32)
            nc.vector.tensor_reduce(out=s2f, in_=e2f, axis=AX.X, op=A.add)
            den = pre.tile([P, SCH], fp32)
            nc.vector.tensor_mul(out=den, in0=s1s, in1=s2f)
            rden = pre.tile([P, SCH], fp32)
            nc.vector.reciprocal(out=rden, in_=den)
            pd = pre.tile([P, SCH], fp32)
            nc.vector.tensor_mul(out=pd, in0=pdsum, in1=rden)
            nc.sync.dma_start(out=pd_d[:, :], in_=pd)

        # ---------------- main loop: [128, 1024] supertiles ----------------
        with tc.tile_pool(name="gpsum", bufs=2, space="PSUM") as gpsum, \
             tc.tile_pool(name="ppsum", bufs=2, space="PSUM") as ppsum, \
             tc.tile_pool(name="work", bufs=3) as work, \
             tc.tile_pool(name="trows", bufs=2) as trows:

            for m in range(SCH):
                trow = trows.tile([P, N], bf16, tag="trow")
                for sup in range(NSUP):
                    idx = m * NSUP + sup
                    tslice = trow[:, sup * SUPW : (sup + 1) * SUPW]
                    g = gpsum.tile([P, SUPW], fp32, tag="g")
                    for kh in range(KH):
                        for h in range(2):
                            nc.tensor.matmul(
                                g[:, h * 512 : (h + 1) * 512],
                                lhsT=fnTs[kh][:, m * P : (m + 1) * P],
                                rhs=fnT[kh][
                                    :,
                                    sup * SUPW + h * 512 : sup * SUPW + (h + 1) * 512,
                                ],
                                start=(kh == 0),
                                stop=(kh == KH - 1),
                            )
                    v = ppsum.tile([P, SUPW], fp32, tag="v")  # v = 2P - 1
                    for h in range(2):
                        nc.tensor.matmul(
                            v[:, h * 512 : (h + 1) * 512],
                            lhsT=p2T[: C + 1, m * P : (m + 1) * P],
                            rhs=p1T[
                                : C + 1,
                                sup * SUPW + h * 512 : sup * SUPW + (h + 1) * 512,
                            ],
                            start=True,
                            stop=True,
                        )
                    # base sum: ln(1 - P + eps) = Ln(-0.5*v + 0.5 + eps)
                    lnq = work.tile([P, SUPW], bf16, tag="lnq")
                    nc.scalar.activation(
                        out=lnq, in_=v, func=F.Ln, bias=bias_ln[:, :],
                        scale=-0.5, accum_out=acc[:, idx : idx + 1],
                    )
                    if idx in ACT_SIGN:
                        # t = Sign(cos - thre) in {+-1}; bf16 out = the target
                        nc.scalar.activation(
                            out=tslice, in_=g, func=F.Sign,
                            bias=bias_sign[:, :], scale=1.0,
                            accum_out=cnt[:, idx : idx + 1],
                        )
                    else:
                        # ind = (cos > thre) in {1,0}; accum (op1=add) counts
                        # positives directly; target = 2*ind - 1
                        hh = work.tile([P, SUPW], bf16, tag="hh")
                        nc.vector.tensor_scalar(
                            out=hh, in0=g, scalar1=COS_THRE, scalar2=None,
                            op0=A.is_gt, op1=A.add,
                            accum_out=cnt[:, idx : idx + 1],
                        )
                        nc.vector.tensor_scalar(
                            out=tslice, in0=hh, scalar1=2.0, scalar2=1.0,
                            op0=A.mult, op1=A.subtract,
                        )
                nc.sync.dma_start(out=tgt[m * P : (m + 1) * P, :], in_=trow)

            nc.sync.dma_start(out=lnacc_d[:, :], in_=acc)
            nc.sync.dma_start(out=cnt_d[:, :], in_=cnt)


def build_nc():
    # Bacc (not plain Bass): its compile() pipeline splits multi-semaphore
    # waits into standalone EventSemaphore instructions, which the walrus
    # per-instruction single-wait-slot limit requires.
    nc = bacc_mod.Bacc()
    with tile.TileContext(nc) as tc:
        _emit(nc, tc)
    nc.finalize()
    return nc


def _get_nc():
    global _NC_CACHE
    if _NC_CACHE is None:
        _NC_CACHE = build_nc()
    return _NC_CACHE


def make_in_maps(f, l1, l2):
    f = np.ascontiguousarray(np.asarray(f, dtype=np.float32))
    l1 = np.ascontiguousarray(np.asarray(l1, dtype=np.float32))
    l2 = np.ascontiguousarray(np.asarray(l2, dtype=np.float32))
    maps = []
    for c in range(NCORES):
        sl = slice(c * R, (c + 1) * R)
        maps.append(
            {
                "f": f,
                "fs": np.ascontiguousarray(f[sl]),
                "l1": l1,
                "l2s": np.ascontiguousarray(l2[sl]),
                "l1s": np.ascontiguousarray(l1[sl]),
                "idt": _IDT,
            }
        )
    return maps


def _numpy_loss(f, l1, l2):
    """Exact fallback (only used if the positive-pair count is not just the
    diagonal, which never happens for this input distribution)."""
    f = np.asarray(f, np.float64)
    fn = f / np.maximum(np.linalg.norm(f, axis=1, keepdims=True), 1e-12)
    p1 = np.exp(l1 - l1.max(axis=1, keepdims=True)).astype(np.float64)
    p1 /= p1.sum(axis=1, keepdims=True)
    p2 = np.exp(l2 - l2.max(axis=1, keepdims=True)).astype(np.float64)
    p2 /= p2.sum(axis=1, keepdims=True)
    total = 0.0
    for c in range(NCORES):
        sl = slice(c * R, (c + 1) * R)
        cos = fn[sl] @ fn.T
        Pm = p2[sl] @ p1.T
        pt = np.where(cos > COS_THRE, Pm, 1.0 - Pm)
        total += np.log(pt + EPS).sum()
    return np.float32(-(total / (N * N)))


def kernel(f, l1, l2, y=None, **_unused):
    global LAST_RESULT
    nc = _get_nc()
    in_maps = make_in_maps(f, l1, l2)
    out = run_bass_kernel_spmd(nc, in_maps, list(range(NCORES)), trace=TRACE)
    LAST_RESULT = out
    results = out.results

    target = np.empty((N, N), np.float32)
    total = 0.0
    count_ok = True
    n_elem = P * SUPW  # elements per supertile
    for c in range(NCORES):
        rs = results[c]
        target[c * R : (c + 1) * R] = np.asarray(rs["tgt"], dtype=np.float32)
        base = np.asarray(rs["lnacc"], np.float64).sum()
        cntm = np.asarray(rs["cnt"], np.float64).sum(axis=0)  # [NTILES]
        pos = 0.0
        for idx in range(NTILES):
            if idx in ACT_SIGN:
                pos += (cntm[idx] + n_elem) / 2.0   # sum of +-1
            else:
                pos += cntm[idx]                    # sum of {1,0}
        pdv = np.asarray(rs["pd"], np.float64)
        if abs(pos - R) > 0.01:
            count_ok = False
            break
        # swap diagonal terms: base had ln(1-P_aa+eps); true is ln(P_aa+eps)
        corr = (np.log(pdv + EPS) - np.log(1.0 - pdv + EPS)).sum()
        total += base + corr

    if not count_ok:
        loss = _numpy_loss(f, l1, l2)
    else:
        loss = np.float32(-(total / float(N * N)))
    return loss, target.reshape(-1)
